# revision 10
# baseline (speedup 1.0000x reference)
"""Trainium2 Bass kernel for nn_DecoderLSTMCell.

Computes, for B=16384 rows:
    gates = y @ W.T + h0 @ U.T + ctx @ C.T + b            # [B, 4H]
    i, f, o, g = split(gates, 4); i,f,o = sigmoid; g = tanh
    c = i * g + f * c0 ; h = o * tanh(c)
Returns (c, h), both [B, H] float32.

Strategy: data-parallel over the batch dim across 8 NeuronCores (2048
rows/core), weights replicated.  The host only re-lays-out data (concat /
transpose / partition-packing, no arithmetic); each core's NEFF does all
math: streams host-packed bf16 operands, runs the fused
[2048 x 4096 x 4096] GEMM on the tensor engine (bf16, fp32 PSUM accum),
and applies the LSTM epilogue on the DVE/ACT engines directly from PSUM.

Per-core loop: 2 batch passes x 8 hidden blocks (e) x 8 row tiles (m);
each (e, m) accumulates 32 matmuls of [K=128]x[M=128]x[N=512] into one
PSUM bank that holds [i|f|o|g] x 128 hidden units for 128 batch rows.
"""

import ml_dtypes
import numpy as np

import concourse.bass as bass
import concourse.tile as tile
from concourse import bacc
import concourse.mybir as mybir
from concourse import bass_utils

P = 128
F32 = mybir.dt.float32
BF16 = mybir.dt.bfloat16
AF = mybir.ActivationFunctionType

# Problem shapes (hardcoded; see module docstring)
B, IN, H, CTX = 16384, 1024, 1024, 2048
KD = IN + H + CTX  # 4096 contraction dim
G = 4 * H
NCORES = 8
BC = B // NCORES  # 2048 batch rows per core
PASSES = 2
CW = 256  # batch column chunk width of the packed x^T layout

LAST_RESULT = None  # BassKernelResults of the most recent run (for test.py)


def build_nc(bc=BC, h=H, kd=KD, passes=PASSES, cw=None, wtb_bufs=3):
    """Build the per-core SPMD Bass module.

    NEFF inputs (host-packed layouts):
      xTh : [bc//cw, P, kd//P, cw] bf16, xTh[ch,p,kt,b] = x[ch*cw+b, kt*P+p]
      wTh : [h//P, P, kd//P, 4P] bf16, wTh[e,p,kt,j*P+u] = Wcat[j*h+e*P+u, kt*P+p]
      c0s : [bc, h] f32
      bb  : [P, 4h] f32, bias broadcast along partitions, grouped like wTh:
            bb[:, e*4P + j*P + u] = b[j*h + e*P + u]
    NEFF outputs: c_out, h_out [bc, h] f32.
    """
    E = h // P
    KT = kd // P
    BPP = bc // passes  # batch rows per pass
    if cw is None:
        cw = min(CW, BPP)
    NCP = BPP // cw  # x^T chunks per pass
    MT = BPP // P  # m tiles per pass
    NW = 4 * P  # psum width: [i|f|o|g] x 128 hidden cols

    nc = bacc.Bacc("TRN2", target_bir_lowering=False)
    xTh = nc.dram_tensor("xTh", (bc // cw, P, KT, cw), BF16, kind="ExternalInput")
    wTh = nc.dram_tensor("wTh", (E, P, KT, NW), BF16, kind="ExternalInput")
    c0s = nc.dram_tensor("c0s", (bc, h), F32, kind="ExternalInput")
    bb = nc.dram_tensor("bb", (P, 4 * h), F32, kind="ExternalInput")
    c_out = nc.dram_tensor("c_out", (bc, h), F32, kind="ExternalOutput")
    h_out = nc.dram_tensor("h_out", (bc, h), F32, kind="ExternalOutput")

    with (
        tile.TileContext(nc) as tc,
        tc.tile_pool(name="xp", bufs=1) as xp,
        tc.tile_pool(name="wp", bufs=wtb_bufs) as wp,
        tc.tile_pool(name="bp", bufs=2) as bp,
        tc.tile_pool(name="cp", bufs=4) as cp,
        tc.tile_pool(name="gp", bufs=3) as gp,
        tc.tile_pool(name="sp", bufs=3) as sp,
        tc.tile_pool(name="pp", bufs=8, space="PSUM") as pp,
    ):
        for p_i in range(passes):
            # x^T for this pass: HWDGE f32 stage load + DVE cast to bf16
            xtb = []
            for mc in range(NCP):
                xt = xp.tile([P, KT, cw], BF16, tag=f"xtb{mc}", name=f"xtb_{p_i}_{mc}")
                xq = max(1, KT // 8) if (p_i == 0 and mc == 0) else max(1, KT // 2)
                for q in range(0, KT, xq):
                    nc.scalar.dma_start(
                        out=xt[:, q:q + xq], in_=xTh[p_i * NCP + mc, :, q:q + xq]
                    )
                xtb.append(xt)
            for e in range(E):
                bias_t = bp.tile([P, NW], F32, tag="bias", name=f"bias_{p_i}_{e}")
                nc.sync.dma_start(out=bias_t[:], in_=bb[:, e * NW:(e + 1) * NW])
                wt = wp.tile([P, KT, NW], BF16, tag="wtb", name=f"wtb_{p_i}_{e}")
                wq = max(1, KT // 8) if (p_i == 0 and e == 0) else max(1, KT // 4)
                for q in range(0, KT, wq):
                    nc.sync.dma_start(out=wt[:, q:q + wq], in_=wTh[e, :, q:q + wq])
                for mp in range(0, MT, 2):
                  pair = []
                  for m in (mp, mp + 1):
                    if m >= MT:
                        continue
                    row0 = p_i * BPP + m * P
                    c0_t = cp.tile([P, P], F32, tag="c0", name=f"c0_{p_i}_{e}_{m}")
                    nc.sync.dma_start(
                        out=c0_t[:], in_=c0s[row0:row0 + P, e * P:(e + 1) * P]
                    )
                    ps = pp.tile([P, NW], F32, tag="ps", name=f"ps_{p_i}_{e}_{m}")
                    pair.append((m, c0_t, ps))
                  for k in range(KT):
                    for m, c0_t, ps in pair:
                        mc, lc = divmod(m * P, cw)
                        nc.tensor.matmul(
                            ps[:],
                            xtb[mc][:, k, lc:lc + P],
                            wt[:, k, :],
                            start=(k == 0),
                            stop=(k == KT - 1),
                        )
                  for m, c0_t, ps in pair:
                    row0 = p_i * BPP + m * P
                    ga = gp.tile([P, NW], F32, tag="ga", name=f"ga_{p_i}_{e}_{m}")
                    nc.vector.tensor_add(ga[:], ps[:], bias_t[:])
                    act = gp.tile([P, NW], F32, tag="act", name=f"act_{p_i}_{e}_{m}")
                    nc.scalar.activation(act[:, 0:3 * P], ga[:, 0:3 * P], AF.Sigmoid)
                    nc.scalar.activation(act[:, 3 * P:4 * P], ga[:, 3 * P:4 * P], AF.Tanh)
                    ct = sp.tile([P, P], F32, tag="ct", name=f"ct_{p_i}_{e}_{m}")
                    nc.vector.tensor_mul(ct[:], act[:, 0:P], act[:, 3 * P:4 * P])
                    fc = sp.tile([P, P], F32, tag="fc", name=f"fc_{p_i}_{e}_{m}")
                    nc.vector.tensor_mul(fc[:], act[:, P:2 * P], c0_t[:])
                    nc.vector.tensor_add(ct[:], ct[:], fc[:])
                    nc.scalar.dma_start(
                        out=c_out[row0:row0 + P, e * P:(e + 1) * P], in_=ct[:]
                    )
                    tct = sp.tile([P, P], F32, tag="tct", name=f"tct_{p_i}_{e}_{m}")
                    nc.scalar.activation(tct[:], ct[:], AF.Tanh)
                    ht = sp.tile([P, P], F32, tag="ht", name=f"ht_{p_i}_{e}_{m}")
                    nc.vector.tensor_mul(ht[:], act[:, 2 * P:3 * P], tct[:])
                    nc.scalar.dma_start(
                        out=h_out[row0:row0 + P, e * P:(e + 1) * P], in_=ht[:]
                    )
    nc.compile()
    return nc


def pack_inputs(y, ctx, c0, h0, W, U, C, b, bc=BC, h=H, kd=KD, cw=CW):
    """Host-side layout packing (pure data movement, no arithmetic)."""
    b_total = y.shape[0]
    E = h // P
    KT = kd // P
    x_all = np.concatenate([y, h0, ctx], axis=1)  # [B, KD]; order matches Wcat
    xTh = np.ascontiguousarray(
        x_all.reshape(b_total // cw, cw, KT, P).transpose(0, 3, 2, 1)
    ).astype(ml_dtypes.bfloat16)
    Wcat = np.concatenate([W, U, C], axis=1)  # [G, KD]
    wTh = np.ascontiguousarray(
        Wcat.reshape(4, E, P, KT, P).transpose(1, 4, 3, 0, 2).reshape(E, P, KT, 4 * P)
    ).astype(ml_dtypes.bfloat16)
    br = b.reshape(4, E, P).transpose(1, 0, 2).reshape(4 * h)
    bb = np.ascontiguousarray(np.broadcast_to(br, (P, 4 * h)))
    return xTh, wTh, bb


def kernel(y, ctx, c0, h0, W, U, C, b):
    global LAST_RESULT
    y = np.ascontiguousarray(np.asarray(y, dtype=np.float32))
    ctx = np.ascontiguousarray(np.asarray(ctx, dtype=np.float32))
    c0 = np.ascontiguousarray(np.asarray(c0, dtype=np.float32))
    h0 = np.ascontiguousarray(np.asarray(h0, dtype=np.float32))
    W = np.ascontiguousarray(np.asarray(W, dtype=np.float32))
    U = np.ascontiguousarray(np.asarray(U, dtype=np.float32))
    C = np.ascontiguousarray(np.asarray(C, dtype=np.float32))
    b = np.ascontiguousarray(np.asarray(b, dtype=np.float32))

    xTh, wTh, bb = pack_inputs(y, ctx, c0, h0, W, U, C, b)

    nc = build_nc()
    cpb = BC // CW  # x^T chunks per core
    in_maps = []
    for c_i in range(NCORES):
        in_maps.append(
            {
                "xTh": xTh[c_i * cpb:(c_i + 1) * cpb],
                "wTh": wTh,
                "c0s": np.ascontiguousarray(c0[c_i * BC:(c_i + 1) * BC]),
                "bb": bb,
            }
        )
    res = bass_utils.run_bass_kernel_spmd(nc, in_maps, core_ids=list(range(NCORES)))
    LAST_RESULT = res
    c_full = np.concatenate([r["c_out"] for r in res.results], axis=0)
    h_full = np.concatenate([r["h_out"] for r in res.results], axis=0)
    return (c_full, h_full)
